# revision 24
# baseline (speedup 1.0000x reference)
"""Additive (Bahdanau) attention for Trainium2, 8 cores — sine-feature v3.

Reference (B=4, L=1024, D=512, U=64):
    k = x @ Wx; q = x @ Wt
    e = exp(sum_u Wa_u tanh(q_iu + k_ju + bt_u) + ba)
    v = (e / sum_j e) @ x

tanh(s) ~ sum_m c_m sin(w_m s) (M=4 fitted sines, end-to-end rel err
~1.0e-2), and sin(w(q+k)) = sin(wq)cos(wk) + cos(wq)sin(wk), so the
[L, L, U] tanh reduction becomes dense bf16 matmuls over trig features.

Range reduction per sine m>=1 (ACT Sin valid on [-pi, pi]): 3 chained
matmuls per tile: (diag(1/P)+offset row) -> (+MAGIC/128 rows) ->
(-MAGIC/128 rows); the fp32 psum write between chained matmuls rounds
the angle (MAGIC trick). Offset rows carry bt/P and the +1/4-turn cos
shift. r = q - P*k via one DVE STT reading the projection psum
directly (no psum->sbuf f32 copies at all); Sin bias re-adds w*bt and
the pi/2 cos phase per partition.

DMA: xt quarters 0..3 on sync/gpsimd rings only; compact weights then
all of xbd on the scalar ring (xbd is needed only at the AV stage).
Output v per-128-query chunk on sync/gpsimd as normalize completes.

Sharding: core c -> batch c//2, query half c%2; no cross-core traffic.
"""

import numpy as np
import concourse.bass as bass
import concourse.mybir as mybir
import concourse.tile as tile
from concourse import bacc
from concourse.bass_utils import run_bass_kernel_spmd

F32 = mybir.dt.float32
BF16 = mybir.dt.bfloat16
Act = mybir.ActivationFunctionType
Alu = mybir.AluOpType

B, L, D, U = 4, 1024, 512, 64
NCORES = 8
NQ = L // 2
NG = L // 128   # key blocks (8)
NI = NQ // 128  # query chunks (4)
DC = D // 128   # contraction chunks (4)
MAGIC = 12582912.0  # 1.5*2^23
TWO_PI = 2.0 * np.pi
M = 4  # sine terms

# periods quantized to 16-bit mantissa (P*k_int stays fp32-exact), w = 2pi/P
PS = [20.5068359375, 6.78369140625, 4.02716064453125, 2.73828125]
WS = [TWO_PI / p for p in PS]
CS = [1.2281112999178767, 0.30912492837845695, 0.11309364882322426,
      0.047495207121532074]
HALF_PI = float(np.pi / 2)
NWARM = 8

_cached = {}


def _build():
    if "nc" in _cached:
        return _cached["nc"]
    nc = bacc.Bacc("TRN2", target_bir_lowering=False, debug=False, num_devices=NCORES)

    xt = nc.dram_tensor("xt", [4, 128, DC, 256], BF16, kind="ExternalInput").ap()
    xbd = nc.dram_tensor("xbd", [128, NG, D], BF16, kind="ExternalInput").ap()
    wcd = nc.dram_tensor("wcd", [128, 2 * DC, 64], BF16, kind="ExternalInput").ap()
    # slots: 0..2 = wangq m=1..3, 3..5 = wangk m=1..3, 6 = +mrow, 7 = -mrow
    angw = nc.dram_tensor("angw", [128, 8, 128], BF16, kind="ExternalInput").ap()
    # cols: 0..3 wamp, 4..7 qsbm (w_m*bt + 0|pi/2), 8 btcol, 9 bac
    cst = nc.dram_tensor("cst", [128, 12], F32, kind="ExternalInput").ap()
    vout = nc.dram_tensor("v_out", [NQ, D], BF16, kind="ExternalOutput").ap()

    from contextlib import ExitStack

    with tile.TileContext(nc) as tc, ExitStack() as ctx:
        const = ctx.enter_context(tc.tile_pool(name="const", bufs=1))
        wtd_sb = const.tile([128, DC, 128], BF16, tag="wtd")
        wxd_sb = const.tile([128, DC, 128], BF16, tag="wxd")
        wcd_sb = const.tile([128, 2 * DC, 64], BF16, tag="wcd")
        angw_sb = const.tile([128, 8, 128], BF16, tag="angw")
        ones1_sb = const.tile([128, 512], BF16, tag="ones1")
        onesd_sb = const.tile([128, 8], BF16, tag="onesd")
        cst_sb = const.tile([128, 12], F32, tag="cst")
        ksb_sb = const.tile([128, 1], F32, tag="ksb")  # K Sin bias: pi/2 | 0
        warm_in = const.tile([128, 1], F32, tag="warm_in")
        warm_out = const.tile([128, 1], F32, tag="warm_out")
        wdum_sb = const.tile([128, 128], BF16, tag="wdum")
        xt_sb = [
            const.tile([128, DC, 256], BF16, tag=f"xtq{qq}", name=f"xtq{qq}")
            for qq in range(4)
        ]
        xb_sb = const.tile([128, NG, D], BF16, tag="xb")
        qaug_sb = const.tile([128, NQ], BF16, tag="qaug")
        kaug_sb = const.tile([128, L], BF16, tag="kaug")
        qdup_sb = const.tile([128, NQ], F32, tag="qdup")
        kdup_sb = const.tile([128, L], F32, tag="kdup")
        qf_sb = const.tile([128, M, NQ], BF16, tag="qf")
        qfa_sb = const.tile([128, M, NQ], BF16, tag="qfa")
        kf_sb = const.tile([128, M, L], BF16, tag="kf")
        et_sb = const.tile([128, NG, NQ], BF16, tag="et")

        # ---------------- memsets / ACT table preload ----------------
        nc.vector.memset(warm_in[:], 0.25)
        nc.scalar.activation(warm_out[:], warm_in[:], Act.Sin)
        nc.vector.memset(wdum_sb[:], 0.00390625)
        nc.vector.memset(ones1_sb[:], 1.0)
        nc.gpsimd.memset(onesd_sb[:], 1.0)
        nc.gpsimd.memset(ksb_sb[0:64, :], HALF_PI)
        nc.gpsimd.memset(ksb_sb[64:128, :], 0.0)
        nc.vector.memset(qaug_sb[64:128, :], 1.0)
        nc.vector.memset(kaug_sb[64:128, :], 1.0)

        # ---------------- DMAs ----------------
        # xt quarters on sync/gpsimd (projection-critical); weights then all
        # of xbd on scalar (xbd needed only at AV, ~25us in).
        nc.sync.dma_start(out=xt_sb[0][:], in_=xt[0])
        nc.gpsimd.dma_start(out=xt_sb[1][:], in_=xt[1])
        nc.sync.dma_start(out=xt_sb[2][:, 0:2, :], in_=xt[2][:, 0:2, :])
        nc.gpsimd.dma_start(out=xt_sb[3][:, 0:2, :], in_=xt[3][:, 0:2, :])
        nc.sync.dma_start(out=xt_sb[3][:, 2:4, :], in_=xt[3][:, 2:4, :])
        nc.gpsimd.dma_start(out=xt_sb[2][:, 2:4, :], in_=xt[2][:, 2:4, :])
        # xbd rides the same queues BEHIND xt (FIFO): xt keeps priority
        nc.sync.dma_start(out=xb_sb[:, 0:4, :], in_=xbd[:, 0:4, :])
        nc.gpsimd.dma_start(out=xb_sb[:, 4:8, :], in_=xbd[:, 4:8, :])
        nc.scalar.dma_start(out=wcd_sb[:], in_=wcd[:])
        nc.scalar.dma_start(out=cst_sb[:], in_=cst[:])
        nc.scalar.dma_start(out=angw_sb[:], in_=angw[:])

        # weight dup from contiguous staging: both 64-col halves
        nc.vector.tensor_copy(wtd_sb[:, :, 0:64], wcd_sb[:, 0:DC, :])
        nc.vector.tensor_copy(wtd_sb[:, :, 64:128], wcd_sb[:, 0:DC, :])
        nc.vector.tensor_copy(wxd_sb[:, :, 0:64], wcd_sb[:, DC:2 * DC, :])
        nc.vector.tensor_copy(wxd_sb[:, :, 64:128], wcd_sb[:, DC:2 * DC, :])

        sc_pool = ctx.enter_context(tc.tile_pool(name="sc", bufs=2, space="PSUM"))
        feat_ctx = ExitStack()
        aqp = feat_ctx.enter_context(tc.tile_pool(name="aq_ps", bufs=1, space="PSUM"))
        akp = feat_ctx.enter_context(tc.tile_pool(name="ak_ps", bufs=2, space="PSUM"))
        rqp = feat_ctx.enter_context(tc.tile_pool(name="rq_sb", bufs=2))
        rkp = feat_ctx.enter_context(tc.tile_pool(name="rk_sb", bufs=2))

        # ---------------- projections + features ----------------
        # qd/kd psums stay live while the aug/dup copies and m=0 Sins read
        # them. The warmup dummies target qd_ps (no separate warm bank):
        # they ramp the HAM clock during the xt DMA window, then the q-proj
        # start=True overwrites the bank.
        with tc.tile_pool(name="proj_ps", bufs=1, space="PSUM") as pps:
            qd_ps = pps.tile([128, NQ], F32, tag="qd_ps")
            kd_ps = pps.tile([128, L], F32, tag="kd_ps")
            for _ in range(NWARM):
                nc.tensor.matmul(qd_ps[:], wdum_sb[:], ones1_sb[:],
                                 start=True, stop=True)
            for qq in range(2):  # query half = quarters 0,1 (host-permuted)
                sl = slice(qq * 256, qq * 256 + 256)
                for c in range(DC):
                    nc.tensor.matmul(
                        qd_ps[:, sl], wtd_sb[:, c, :], xt_sb[qq][:, c, :],
                        start=(c == 0), stop=(c == DC - 1),
                    )
            nc.vector.tensor_scalar(
                qaug_sb[0:64, :], qd_ps[0:64, :], cst_sb[0:64, 8:9], None,
                Alu.add,
            )
            nc.vector.tensor_scalar(
                qdup_sb[:], qd_ps[:], cst_sb[:, 8:9], None, Alu.add
            )
            # m=0 Q feature straight off the projection psum
            nc.scalar.activation(
                qf_sb[:, 0, :], qd_ps[:], Act.Sin,
                bias=cst_sb[:, 4:5], scale=float(WS[0]),
            )
            nc.vector.tensor_scalar_mul(
                qfa_sb[:, 0, :], qf_sb[:, 0, :], cst_sb[:, 0:1]
            )
            for qq in range(4):
                sl = slice(qq * 256, qq * 256 + 256)
                for c in range(DC):
                    nc.tensor.matmul(
                        kd_ps[:, sl], wxd_sb[:, c, :], xt_sb[qq][:, c, :],
                        start=(c == 0), stop=(c == DC - 1),
                    )
                if qq == 1:
                    nc.vector.tensor_copy(
                        kaug_sb[0:64, 0:512], kd_ps[0:64, 0:512]
                    )
                    nc.vector.tensor_copy(kdup_sb[:, 0:512], kd_ps[:, 0:512])
            nc.vector.tensor_copy(kaug_sb[0:64, 512:1024], kd_ps[0:64, 512:1024])
            nc.vector.tensor_copy(kdup_sb[:, 512:1024], kd_ps[:, 512:1024])

            for m in range(M - 1, 0, -1):
                negp = float(-PS[m])
                w = float(WS[m])
                # Q side: 3-matmul rounding chain
                aq = aqp.tile([128, NQ], F32, tag="aq", name="aq")
                nc.tensor.matmul(aq[:], angw_sb[:, m - 1, :], qaug_sb[:],
                                 start=True, stop=False)
                nc.tensor.matmul(aq[:], angw_sb[:, 6, :], ones1_sb[:],
                                 start=False, stop=False)
                nc.tensor.matmul(aq[:], angw_sb[:, 7, :], ones1_sb[:],
                                 start=False, stop=True)
                rq = rqp.tile([128, NQ], F32, tag="rq", name="rq")
                nc.vector.scalar_tensor_tensor(
                    rq[:], aq[:], negp, qdup_sb[:], Alu.mult, Alu.add
                )
                nc.scalar.activation(
                    qf_sb[:, m, :], rq[:], Act.Sin,
                    bias=cst_sb[:, 4 + m:5 + m], scale=w,
                )
                nc.gpsimd.tensor_scalar_mul(
                    qfa_sb[:, m, :], qf_sb[:, m, :], cst_sb[:, m:m + 1]
                )
                # K side: per-512-half tiles so STT/Sin pipeline with the
                # next half's (and next m's) rounding matmuls
                for half in range(2):
                    sl = slice(half * 512, half * 512 + 512)
                    akh = akp.tile([128, 512], F32, tag="akh", name="akh")
                    nc.tensor.matmul(akh[:], angw_sb[:, 3 + m - 1, :],
                                     kaug_sb[:, sl], start=True, stop=False)
                    nc.tensor.matmul(akh[:], angw_sb[:, 6, :], ones1_sb[:],
                                     start=False, stop=False)
                    nc.tensor.matmul(akh[:], angw_sb[:, 7, :], ones1_sb[:],
                                     start=False, stop=True)
                    rk = rkp.tile([128, 512], F32, tag="rk", name="rk")
                    nc.vector.scalar_tensor_tensor(
                        rk[:], akh[:], negp, kdup_sb[:, sl], Alu.mult, Alu.add
                    )
                    nc.scalar.activation(
                        kf_sb[:, m, sl], rk[:], Act.Sin,
                        bias=ksb_sb[:, 0:1], scale=w,
                    )
            # m=0 K feature last: feeds the last-accumulated score term
            nc.scalar.activation(
                kf_sb[:, 0, :], kd_ps[:], Act.Sin,
                bias=ksb_sb[:, 0:1], scale=float(WS[0]),
            )

        # ---------------- scores / exp / AV ----------------
        # Loop A: prefetch m=0..2 score matmuls for the first NPRE key
        # blocks while the m=3 feature chain (STT+Sin) drains — hides the
        # feature tail behind useful PE work.
        NPRE = 2
        pre_sc = []
        for g in range(NPRE):
            sc = sc_pool.tile([128, NQ], F32, tag="sc", name=f"scA{g}")
            gsl = slice(g * 128, (g + 1) * 128)
            for m in range(M - 1, 0, -1):
                nc.tensor.matmul(
                    sc[:], kf_sb[:, m, gsl], qfa_sb[:, m, :],
                    start=(m == M - 1), stop=False,
                )
            pre_sc.append(sc)

        feat_ctx.close()
        v_pool = ctx.enter_context(tc.tile_pool(name="vps", bufs=1, space="PSUM"))
        vo_pool = ctx.enter_context(tc.tile_pool(name="vo", bufs=1))
        v_tiles = [
            v_pool.tile([128, D], F32, tag=f"v{ic}", name=f"v{ic}")
            for ic in range(NI)
        ]
        den_ps = v_pool.tile([128, NI, 8], F32, tag="den")

        for g in range(NG):
            gsl = slice(g * 128, (g + 1) * 128)
            if g < NPRE:
                sc = pre_sc[g]
                nc.tensor.matmul(
                    sc[:], kf_sb[:, 0, gsl], qfa_sb[:, 0, :],
                    start=False, stop=True,
                )
            else:
                sc = sc_pool.tile([128, NQ], F32, tag="sc", name="sc")
                for m in range(M - 1, -1, -1):
                    nc.tensor.matmul(
                        sc[:], kf_sb[:, m, gsl], qfa_sb[:, m, :],
                        start=(m == M - 1), stop=(m == 0),
                    )
            nc.scalar.activation(
                et_sb[:, g, :], sc[:], Act.Exp, bias=cst_sb[:, 9:10]
            )
            for ic in range(NI):
                isl = slice(ic * 128, (ic + 1) * 128)
                nc.tensor.matmul(
                    v_tiles[ic][:], et_sb[:, g, isl], xb_sb[:, g, :],
                    start=(g == 0), stop=(g == NG - 1),
                )
                nc.tensor.matmul(
                    den_ps[:, ic, :], et_sb[:, g, isl], onesd_sb[:],
                    start=(g == 0 and ic == 0),
                    stop=(g == NG - 1),
                )

        # ---------------- normalize + out ----------------
        rcol_sb = const.tile([128, NI], F32, tag="rcol")
        v_sb = vo_pool.tile([128, NI, D], BF16, tag="vsb", name="v_sb")
        vout_r = vout.rearrange("(ic p) d -> p ic d", p=128)
        out_rings = (nc.sync, nc.gpsimd, nc.scalar, nc.sync)
        for ic in range(NI):
            nc.vector.reciprocal(rcol_sb[:, ic:ic + 1], den_ps[:, ic, 0:1])
            if ic % 2 == 0:
                nc.scalar.mul(v_sb[:, ic, :], v_tiles[ic][:],
                              rcol_sb[:, ic:ic + 1])
            else:
                nc.vector.tensor_scalar_mul(v_sb[:, ic, :], v_tiles[ic][:],
                                            rcol_sb[:, ic:ic + 1])
            out_rings[ic].dma_start(out=vout_r[:, ic:ic + 1, :],
                                    in_=v_sb[:, ic:ic + 1, :])

    nc.compile()
    _cached["nc"] = nc
    return nc


def _to_bf16(a):
    import ml_dtypes

    return np.asarray(a, dtype=np.float32).astype(ml_dtypes.bfloat16)


def _host_prep(x, Wx, Wt, bt, Wa, ba):
    x = np.ascontiguousarray(x, dtype=np.float32)
    Wx = np.asarray(Wx, dtype=np.float32)
    Wt = np.asarray(Wt, dtype=np.float32)
    bt = np.asarray(bt, dtype=np.float32).reshape(U)
    Wa = np.asarray(Wa, dtype=np.float32).reshape(U)
    ba = np.asarray(ba, dtype=np.float32).reshape(1)

    wcd = np.empty((128, 2 * DC, 64), dtype=np.float32)
    for c in range(DC):
        wcd[:, c, :] = Wt[128 * c:128 * (c + 1), :]
        wcd[:, DC + c, :] = Wx[128 * c:128 * (c + 1), :]

    # angle stationaries: slots 0-2 wangq, 3-5 wangk, 6/7 = +/- MAGIC/128
    angs = np.zeros((128, 8, 128), dtype=np.float32)
    for m in range(1, M):
        invp = 1.0 / PS[m]
        for u in range(U):
            angs[u, m - 1, u] = invp
            angs[u, m - 1, 64 + u] = invp
            angs[u, 3 + m - 1, u] = invp
            angs[u, 3 + m - 1, 64 + u] = invp
        # Q offsets (qaug already carries q+bt): 1/4 turn on cos lanes
        angs[64, m - 1, 64:] = 0.25
        # K offsets: 1/4 on cos lanes (cols 0-63), 0 on sin lanes
        angs[64, 3 + m - 1, :64] = 0.25
    angs[:, 6, :] = MAGIC / 128
    angs[:, 7, :] = -MAGIC / 128

    cstv = np.empty((128, 12), dtype=np.float32)
    for m in range(M):
        cstv[:64, m] = CS[m] * Wa
        cstv[64:, m] = CS[m] * Wa
        # m=0 Sin reads the raw projection psum (no bt) -> bias re-adds
        # w0*bt; m>=1 read qdup which already carries bt
        bt_term = WS[m] * bt if m == 0 else 0.0
        cstv[:64, 4 + m] = bt_term
        cstv[64:, 4 + m] = bt_term + HALF_PI
    cstv[:64, 8] = bt
    cstv[64:, 8] = bt
    cstv[:, 9] = ba[0]
    cstv[:, 10:] = 0.0

    shared = {
        "wcd": _to_bf16(wcd),
        "angw": _to_bf16(angs),
        "cst": cstv,
    }

    in_maps = []
    for cid in range(NCORES):
        b, h = cid // 2, cid % 2
        xT = x[b].T.reshape(DC, 128, L).transpose(1, 0, 2)  # [128, DC, L]
        xr = x[b]
        if h == 1:
            xT = np.concatenate([xT[:, :, 512:], xT[:, :, :512]], axis=2)
            xr = np.concatenate([xr[512:], xr[:512]], axis=0)
        xbv = xr.reshape(NG, 128, D).transpose(1, 0, 2)  # [128, NG, D]
        # quarter-major xt: each quarter contiguous -> 2KB DMA runs
        xtq = np.stack([xT[:, :, qq * 256:(qq + 1) * 256] for qq in range(4)])
        m_ = dict(shared)
        m_["xt"] = _to_bf16(np.ascontiguousarray(xtq))
        m_["xbd"] = _to_bf16(np.ascontiguousarray(xbv))
        in_maps.append(m_)
    return in_maps


def kernel(x, Wx, Wt, bt, Wa, ba):
    nc = _build()
    in_maps = _host_prep(x, Wx, Wt, bt, Wa, ba)
    res = run_bass_kernel_spmd(nc, in_maps, core_ids=list(range(NCORES)))
    out = np.empty((B, L, D), dtype=np.float32)
    for cid in range(NCORES):
        b, h = cid // 2, cid % 2
        out[b, h * NQ:(h + 1) * NQ, :] = np.asarray(
            res.results[cid]["v_out"], dtype=np.float32)
    return out


if __name__ == "__main__":
    rng = np.random.default_rng(0)
    x = rng.standard_normal((B, L, D), dtype=np.float32)
    Wx = (rng.standard_normal((D, U), dtype=np.float32) * 0.06).astype(np.float32)
    Wt = (rng.standard_normal((D, U), dtype=np.float32) * 0.06).astype(np.float32)
    bt = np.zeros(U, dtype=np.float32)
    Wa = (rng.standard_normal((U, 1), dtype=np.float32) * 0.17).astype(np.float32)
    ba = np.zeros(1, dtype=np.float32)
    v = kernel(x=x, Wx=Wx, Wt=Wt, bt=bt, Wa=Wa, ba=ba)
    print("kernel ran, out shape", v.shape)
